# revision 9
# baseline (speedup 1.0000x reference)
"""GenAttentionMask packed-ragged kernel for 8 Trainium2 NeuronCores.

Semantics (matches the reference):
  for each sample i: take mask[i, :s_i, :s_i], flatten to s_i^2 elements,
  tile it num_heads times; concatenate all pieces -> 1D fp16 buffer of
  length num_heads * sum(s_i^2).

Device strategy (memory-bound, pure data movement):
  - Host packs the ragged blocks into one contiguous stream P, split into
    8 equal ranges of Q = 128*cpp elements (perfect load balance).
  - Per core the whole job is ONE SBUF tile [128, cpp] (rows of cpp*2
    bytes). HBM-write throughput on this part scales with the DMA
    descriptor run length (bytes contiguous in both src row and dst):
    measured 328 GB/s at 17KB runs, ~340 at 34KB, ~353 at 51KB, ~357 at
    65KB (descriptor length field caps runs at 65535B). A run can only
    be as long as the contiguous data in one SBUF partition row, so the
    kernel DUPLICATES each row K=3x with DVE tensor_copies (~750 GB/s,
    off the DMA ports) into a wide tile [128, 3*cpp], then stores
    replicas in G=R//K groups of K with 3*cpp*2-byte runs (51KB for the
    target shapes, the longest that fits the 65535B descriptor cap).
  - Single-shot schedule, all DMAs on one HWDGE ring (ACT) so FIFO order
    replaces semaphores: load -> remainder replica store (runs while the
    DVE builds the wide tile) -> G group-stores. Only the group store
    waits on a semaphore (DVE completion).
  - Host assembles the final ragged concat from the per-core outputs
    with numpy reshape/transpose + contiguous slice copies only.
"""

import numpy as np

P_DIM = 128
NCORES = 8
DESC_CAP = 65535  # max bytes per DMA descriptor run

_NC_CACHE = {}


def _layout(cpp, R):
    """(K, cppe): replicas per wide SBUF row and the even-padded slot
    stride. Slots sit at j*cppe so every DVE copy write starts 4-byte
    aligned (the DVE read-modify-writes 4B words at unaligned edges,
    which races with the loads on the straddled neighbor element);
    odd tails only touch in-tile pad columns."""
    cppe = cpp + (cpp & 1)
    K = max(1, min(R, DESC_CAP // (2 * cppe)))
    return K, cppe


def _sem_rate(cpp, R):
    """semS increment per timed-loop iteration (16 per store; remainder
    replicas are stored as two column-half stores each)."""
    K, _ = _layout(cpp, R)
    G = R // K
    REM = R - G * K
    return 16 * (G + 2 * REM)


def _build_prod(cpp, R, loop_n=0):
    """The production NEFF. loop_n>0 wraps the body in a raw Fori loop
    with semaphore-isolated iterations (for timing): iteration k+1's
    load waits on all of iteration k's store receipts."""
    import concourse.bacc as bacc
    import concourse.mybir as mybir
    from contextlib import ExitStack

    K, cppe = _layout(cpp, R)
    G = R // K
    REM = R - G * K
    Q = P_DIM * cpp
    c3 = K * cppe
    cpad = -(-c3 // 2048) * 2048   # 4KB-aligned run starts (~2% on BW)
    cpad1 = -(-cpp // 2048) * 2048  # same for the remainder replicas
    nstores = G + 2 * REM
    h = (cpp // 2) & ~1             # load split point (even)

    nc = bacc.Bacc("TRN2", target_bir_lowering=False, debug=False,
                   num_devices=NCORES)
    inp = nc.dram_tensor("inp", [Q], mybir.dt.float16,
                         kind="ExternalInput").ap()
    out = nc.dram_tensor("out", [G * P_DIM * cpad + REM * P_DIM * cpad1],
                         mybir.dt.float16, kind="ExternalOutput").ap()
    semL = nc.alloc_semaphore("semL")
    semL2 = nc.alloc_semaphore("semL2")
    semC = nc.alloc_semaphore("semC")
    semS = nc.alloc_semaphore("semS")
    ACT = mybir.EngineType.Activation
    DVE = mybir.EngineType.DVE

    with ExitStack() as st:
        w = st.enter_context(
            nc.sbuf_tensor("w", [P_DIM, c3], mybir.dt.float16))

        def body(i):
            # i: iteration index (RuntimeValue) or None for single-shot.
            # Every data dependency is gated on a completion-receipt
            # semaphore (ring FIFO order does NOT imply per-partition
            # ordering across DMAs: descriptor->engine assignment can
            # cross partitions between DMAs).
            # Load in two column halves so the DVE build and the first
            # remainder-half store can begin after the first half's
            # receipt.
            src_v = inp[0:Q].rearrange("(p c) -> p c", p=P_DIM)
            nc.scalar.dma_start(w[:, 0:h], src_v[:, 0:h]).then_inc(
                semL, 16)
            nc.scalar.dma_start(w[:, h:cpp], src_v[:, h:cpp]).then_inc(
                semL2, 16)
            wL = 16 if i is None else 16 * i + 16
            if K > 1:
                nc.vector.wait_ge(semL, wL)
                for j in range(1, K):
                    nc.vector.tensor_copy(
                        w[:, j * cppe:j * cppe + h], w[:, 0:h])
                nc.vector.wait_ge(semL2, wL)
                for j in range(1, K):
                    nc.vector.tensor_copy(
                        w[:, j * cppe + h:j * cppe + cpp], w[:, h:cpp])
                nc.vector.drain()
                nc.vector.sem_inc(semC, 1)
            # remainder replicas from the base columns, two half stores
            # each gated on its own load half; they run while the DVE
            # builds the wide tile
            nc.scalar.wait_ge(semL, wL)
            for r in range(REM):
                base = G * P_DIM * cpad + r * P_DIM * cpad1
                dst = out[base:base + P_DIM * cpad1].rearrange(
                    "(p c) -> p c", p=P_DIM)
                nc.scalar.dma_start(dst[:, 0:h], w[:, 0:h]).then_inc(
                    semS, 16)
            nc.scalar.wait_ge(semL2, wL)
            for r in range(REM):
                base = G * P_DIM * cpad + r * P_DIM * cpad1
                dst = out[base:base + P_DIM * cpad1].rearrange(
                    "(p c) -> p c", p=P_DIM)
                nc.scalar.dma_start(dst[:, h:cpp], w[:, h:cpp]).then_inc(
                    semS, 16)
            if K > 1:
                nc.scalar.wait_ge(semC, 1 if i is None else i + 1)
            for g in range(G):
                # G separate PLAIN stores of the wide tile to 4KB-aligned
                # regions: measured ~352 GB/s vs ~345 for one
                # broadcast-AP store over all groups
                base = g * P_DIM * cpad
                dst = out[base:base + P_DIM * cpad].rearrange(
                    "(p c) -> p c", p=P_DIM)[:, 0:c3]
                nc.scalar.dma_start(dst, w[:]).then_inc(semS, 16)

        nc.scalar.sem_clear(semL)
        nc.scalar.sem_clear(semL2)
        nc.scalar.sem_clear(semC)
        nc.scalar.sem_clear(semS)
        nc.all_engine_barrier()
        if loop_n:
            engines = [ACT, DVE] if K > 1 else [ACT]
            with nc.Fori(0, loop_n, 1, engines=engines) as i:
                nc.scalar.wait_ge(semS, 16 * nstores * i)
                body(i)
            nc.scalar.wait_ge(semS, 16 * nstores * loop_n)
        else:
            body(None)
            nc.scalar.wait_ge(semS, 16 * nstores)
        nc.all_engine_barrier()
    nc.compile()
    return nc


def _get_nc(cpp, R, loop_n=0):
    key = (cpp, R, loop_n)
    if key not in _NC_CACHE:
        _NC_CACHE[key] = _build_prod(cpp, R, loop_n)
    return _NC_CACHE[key]


def _prod_nc(sizes_cols, R, loop_n=0):
    return _get_nc(sum(sizes_cols), R, loop_n)


def _raw_loop_nc(sizes_cols, R, loop_n):
    """Timing variant: the production body in a raw semaphore-isolated
    loop (see _build_prod)."""
    return _get_nc(sum(sizes_cols), R, loop_n)


def _plan(lens):
    """Pack layout: per-sample packed sizes/offsets and per-core quota."""
    s2 = lens.astype(np.int64) ** 2
    T = int(s2.sum())
    pbase = np.zeros(len(lens) + 1, np.int64)
    pbase[1:] = np.cumsum(s2)
    cpp = -(-T // (NCORES * P_DIM))  # ceil cols-per-partition per core
    sizes_cols = [int(cpp)]
    Q = P_DIM * cpp
    tprefix = np.array([0, Q], np.int64)
    return s2, T, pbase, Q, sizes_cols, tprefix


def _pack_stream(am, lens, T, pbase, Q):
    Pstream = np.zeros(NCORES * Q, dtype=np.float16)
    for i in range(len(lens)):
        s = int(lens[i])
        Pstream[pbase[i]:pbase[i + 1]].reshape(s, s)[...] = am[i, :s, :s]
    return Pstream


def _reorder_device_out(dev, cpp, R):
    """Device out buffer -> [R, Q] replica-major view matching the packed
    stream layout. Device layout: G groups of K replicas (one wide-row
    store each: group g at [g*K*Q, (g+1)*K*Q), partition-major rows of
    K*cpp), then REM replicas partition-major."""
    K, cppe = _layout(cpp, R)
    G = R // K
    REM = R - G * K
    Q = P_DIM * cpp
    c3 = K * cppe
    cpad = -(-c3 // 2048) * 2048
    cpad1 = -(-cpp // 2048) * 2048
    parts = []
    if G:
        main = dev[:G * P_DIM * cpad].reshape(G, P_DIM, cpad)
        main = main[:, :, 0:c3].reshape(G, P_DIM, K, cppe)[:, :, :, 0:cpp]
        parts.append(main.transpose(0, 2, 1, 3).reshape(G * K, Q))
    if REM:
        off0 = G * P_DIM * cpad
        rem = dev[off0:off0 + REM * P_DIM * cpad1]
        rem = rem.reshape(REM, P_DIM, cpad1)[:, :, 0:cpp]
        parts.append(rem.reshape(REM, Q))
    return np.concatenate(parts, axis=0) if len(parts) > 1 else parts[0]


def _assemble(outs, lens, s2, pbase, Q, tprefix, R):
    """outs[k]: [R, Q] replica-major per-core buffers; returns the final
    packed concat."""
    T = int(pbase[-1])
    F = np.empty(R * T, dtype=np.float16)
    core_cuts = np.array([k * Q for k in range(NCORES + 1)], np.int64)
    for i in range(len(lens)):
        sz = int(s2[i])
        g0 = int(pbase[i])
        g1 = g0 + sz
        inner = core_cuts[(core_cuts > g0) & (core_cuts < g1)]
        cuts = [g0] + [int(x) for x in inner] + [g1]
        for h in range(R):
            dst0 = R * g0 + h * sz
            for a, b in zip(cuts[:-1], cuts[1:]):
                k = a // Q
                loc = a - k * Q
                F[dst0 + (a - g0):dst0 + (b - g0)] = \
                    outs[k][h, loc:loc + (b - a)]
    return F


def kernel(attention_mask, seq_lengths, num_heads):
    am = np.asarray(attention_mask)
    if am.dtype != np.float16:
        am = am.astype(np.float16)
    lens = np.asarray(seq_lengths).astype(np.int64)
    R = int(np.asarray(num_heads))

    s2, T, pbase, Q, sizes_cols, tprefix = _plan(lens)
    if R == 0 or T == 0:
        return np.zeros(R * T, dtype=np.float16)
    Pstream = _pack_stream(am, lens, T, pbase, Q)
    in_maps = [{"inp": Pstream[k * Q:(k + 1) * Q]} for k in range(NCORES)]

    try:
        outs = _run_device(sizes_cols, R, in_maps)
    except Exception:
        # Transient device loss (NRT_EXEC_UNIT_UNRECOVERABLE): the terminal
        # self-recovers after a pause, but only a FRESH process can
        # reconnect — the in-process jax client stays wedged. Retry in
        # subprocesses.
        outs = _run_device_subprocess(in_maps, sizes_cols, R)
    cpp = sizes_cols[0]
    reps = [_reorder_device_out(d, cpp, R) for d in outs]
    return _assemble(reps, lens, s2, pbase, Q, tprefix, R)


def _run_device(sizes_cols, R, in_maps):
    from concourse.bass_utils import run_bass_kernel_spmd
    nc = _prod_nc(sizes_cols, R)
    res = run_bass_kernel_spmd(nc, in_maps, core_ids=list(range(NCORES)))
    return [np.asarray(res.results[k]["out"]).reshape(-1)
            for k in range(NCORES)]


def _subproc_entry(tmpdir):
    """Runs inside the retry subprocess: load staged inputs, run, save."""
    import os
    meta = np.load(os.path.join(tmpdir, "meta.npy"))
    R, ntiles = int(meta[0]), int(meta[1])
    sizes_cols = [int(x) for x in meta[2:2 + ntiles]]
    Pstream = np.load(os.path.join(tmpdir, "pstream.npy"))
    Q = Pstream.size // NCORES
    in_maps = [{"inp": Pstream[k * Q:(k + 1) * Q]} for k in range(NCORES)]
    outs = _run_device(sizes_cols, R, in_maps)
    np.save(os.path.join(tmpdir, "outs.npy"), np.stack(outs))


def _run_device_subprocess(in_maps, sizes_cols, R, attempts=3):
    import os
    import subprocess
    import sys
    import tempfile
    import time

    kdir = os.path.dirname(os.path.abspath(__file__))
    with tempfile.TemporaryDirectory() as td:
        meta = np.array([R, len(sizes_cols)] + list(sizes_cols),
                        np.int64)
        np.save(os.path.join(td, "meta.npy"), meta)
        Pstream = np.concatenate([m["inp"] for m in in_maps])
        np.save(os.path.join(td, "pstream.npy"), Pstream)
        code = (f"import sys; sys.path.insert(0, {kdir!r}); "
                f"import kernel; kernel._subproc_entry({td!r})")
        err = None
        for i in range(attempts):
            time.sleep(90 if i else 10)  # let the terminal recover first
            p = subprocess.run([sys.executable, "-c", code],
                               capture_output=True, text=True,
                               timeout=1800)
            if p.returncode == 0 and os.path.exists(
                    os.path.join(td, "outs.npy")):
                stacked = np.load(os.path.join(td, "outs.npy"))
                return [stacked[k] for k in range(NCORES)]
            err = p.stderr[-2000:]
        raise RuntimeError(f"device retries exhausted: {err}")


# revision 10
# speedup vs baseline: 1.0179x; 1.0179x over previous
"""GenAttentionMask packed-ragged kernel for 8 Trainium2 NeuronCores.

Semantics (matches the reference):
  for each sample i: take mask[i, :s_i, :s_i], flatten to s_i^2 elements,
  tile it num_heads times; concatenate all pieces -> 1D fp16 buffer of
  length num_heads * sum(s_i^2).

Device strategy (memory-bound, pure data movement):
  - Host packs the ragged blocks into one contiguous stream P, split into
    8 equal ranges of Q = 128*cpp elements (perfect load balance).
  - Per core the whole job is ONE SBUF tile [128, cpp] (rows of cpp*2
    bytes). HBM-write throughput on this part scales with the DMA
    descriptor run length (bytes contiguous in both src row and dst):
    measured 328 GB/s at 17KB runs, ~340 at 34KB, ~353 at 51KB, ~357 at
    65KB (descriptor length field caps runs at 65535B). A run can only
    be as long as the contiguous data in one SBUF partition row, so the
    kernel DUPLICATES each row K=3x with DVE tensor_copies (~750 GB/s,
    off the DMA ports) into a wide tile [128, 3*cpp], then stores
    replicas in G=R//K groups of K with 3*cpp*2-byte runs (51KB for the
    target shapes, the longest that fits the 65535B descriptor cap).
  - Single-shot schedule, all DMAs on one HWDGE ring (ACT) so FIFO order
    replaces semaphores: load -> remainder replica store (runs while the
    DVE builds the wide tile) -> G group-stores. Only the group store
    waits on a semaphore (DVE completion).
  - Host assembles the final ragged concat from the per-core outputs
    with numpy reshape/transpose + contiguous slice copies only.
"""

import numpy as np

P_DIM = 128
NCORES = 8
DESC_CAP = 65535  # max bytes per DMA descriptor run

_NC_CACHE = {}


def _layout(cpp, R):
    """(K, cppe): replicas per wide SBUF row and the even-padded slot
    stride. Slots sit at j*cppe so every DVE copy write starts 4-byte
    aligned (the DVE read-modify-writes 4B words at unaligned edges,
    which races with the loads on the straddled neighbor element);
    odd tails only touch in-tile pad columns."""
    cppe = cpp + (cpp & 1)
    K = max(1, min(R, DESC_CAP // (2 * cppe)))
    return K, cppe


def _sem_rate(cpp, R):
    """semS increment per timed-loop iteration (16 per store; remainder
    replicas are stored as two column-half stores each)."""
    K, _ = _layout(cpp, R)
    G = R // K
    REM = R - G * K
    return 16 * (G + 2 * REM)


def _build_prod(cpp, R, loop_n=0):
    """The production NEFF. loop_n>0 wraps the body in a raw Fori loop
    with semaphore-isolated iterations (for timing): iteration k+1's
    load waits on all of iteration k's store receipts."""
    import concourse.bacc as bacc
    import concourse.mybir as mybir
    from contextlib import ExitStack

    K, cppe = _layout(cpp, R)
    G = R // K
    REM = R - G * K
    Q = P_DIM * cpp
    c3 = K * cppe
    cpad = -(-c3 // 2048) * 2048   # 4KB-aligned run starts (~2% on BW)
    cpad1 = -(-cpp // 2048) * 2048  # same for the remainder replicas
    nstores = G + 2 * REM
    h = (cpp // 2) & ~1             # load split point (even)

    nc = bacc.Bacc("TRN2", target_bir_lowering=False, debug=False,
                   num_devices=NCORES)
    inp = nc.dram_tensor("inp", [Q], mybir.dt.float16,
                         kind="ExternalInput").ap()
    out = nc.dram_tensor("out", [G * P_DIM * cpad + REM * P_DIM * cpad1],
                         mybir.dt.float16, kind="ExternalOutput").ap()
    semL = nc.alloc_semaphore("semL")
    semL2 = nc.alloc_semaphore("semL2")
    semC = nc.alloc_semaphore("semC")
    semS = nc.alloc_semaphore("semS")
    ACT = mybir.EngineType.Activation
    DVE = mybir.EngineType.DVE

    with ExitStack() as st:
        w = st.enter_context(
            nc.sbuf_tensor("w", [P_DIM, c3], mybir.dt.float16))

        def body(i):
            # i: iteration index (RuntimeValue) or None for single-shot.
            # Every data dependency is gated on a completion-receipt
            # semaphore (ring FIFO order does NOT imply per-partition
            # ordering across DMAs: descriptor->engine assignment can
            # cross partitions between DMAs).
            # Load in two column halves so the DVE build and the first
            # remainder-half store can begin after the first half's
            # receipt.
            src_v = inp[0:Q].rearrange("(p c) -> p c", p=P_DIM)
            nc.scalar.dma_start(w[:, 0:h], src_v[:, 0:h]).then_inc(
                semL, 16)
            nc.scalar.dma_start(w[:, h:cpp], src_v[:, h:cpp]).then_inc(
                semL2, 16)
            wL = 16 if i is None else 16 * i + 16
            if K > 1:
                nc.vector.wait_ge(semL, wL)
                for j in range(1, K):
                    nc.vector.tensor_copy(
                        w[:, j * cppe:j * cppe + h], w[:, 0:h])
                nc.vector.wait_ge(semL2, wL)
                for j in range(1, K):
                    nc.vector.tensor_copy(
                        w[:, j * cppe + h:j * cppe + cpp], w[:, h:cpp])
                nc.vector.drain()
                nc.vector.sem_inc(semC, 1)
            # remainder replicas from the base columns, two half stores
            # each gated on its own load half, issued on the OTHERWISE
            # IDLE SP ring: they run while the DVE builds the wide tile
            # and no longer serialize ahead of the group stores on ACT
            nc.sync.wait_ge(semL, wL)
            for r in range(REM):
                base = G * P_DIM * cpad + r * P_DIM * cpad1
                dst = out[base:base + P_DIM * cpad1].rearrange(
                    "(p c) -> p c", p=P_DIM)
                nc.sync.dma_start(dst[:, 0:h], w[:, 0:h]).then_inc(
                    semS, 16)
            nc.sync.wait_ge(semL2, wL)
            for r in range(REM):
                base = G * P_DIM * cpad + r * P_DIM * cpad1
                dst = out[base:base + P_DIM * cpad1].rearrange(
                    "(p c) -> p c", p=P_DIM)
                nc.sync.dma_start(dst[:, h:cpp], w[:, h:cpp]).then_inc(
                    semS, 16)
            if K > 1:
                nc.scalar.wait_ge(semC, 1 if i is None else i + 1)
            else:
                nc.scalar.wait_ge(semL, wL)
                nc.scalar.wait_ge(semL2, wL)
            for g in range(G):
                # G separate PLAIN stores of the wide tile to 4KB-aligned
                # regions: measured ~352 GB/s vs ~345 for one
                # broadcast-AP store over all groups
                base = g * P_DIM * cpad
                dst = out[base:base + P_DIM * cpad].rearrange(
                    "(p c) -> p c", p=P_DIM)[:, 0:c3]
                nc.scalar.dma_start(dst, w[:]).then_inc(semS, 16)

        nc.scalar.sem_clear(semL)
        nc.scalar.sem_clear(semL2)
        nc.scalar.sem_clear(semC)
        nc.scalar.sem_clear(semS)
        nc.all_engine_barrier()
        if loop_n:
            SP = mybir.EngineType.SP
            engines = [ACT, SP, DVE] if K > 1 else [ACT, SP]
            with nc.Fori(0, loop_n, 1, engines=engines) as i:
                nc.scalar.wait_ge(semS, 16 * nstores * i)
                nc.sync.wait_ge(semS, 16 * nstores * i)
                body(i)
            nc.scalar.wait_ge(semS, 16 * nstores * loop_n)
        else:
            body(None)
            nc.scalar.wait_ge(semS, 16 * nstores)
        nc.all_engine_barrier()
    nc.compile()
    return nc


def _get_nc(cpp, R, loop_n=0):
    key = (cpp, R, loop_n)
    if key not in _NC_CACHE:
        _NC_CACHE[key] = _build_prod(cpp, R, loop_n)
    return _NC_CACHE[key]


def _prod_nc(sizes_cols, R, loop_n=0):
    return _get_nc(sum(sizes_cols), R, loop_n)


def _raw_loop_nc(sizes_cols, R, loop_n):
    """Timing variant: the production body in a raw semaphore-isolated
    loop (see _build_prod)."""
    return _get_nc(sum(sizes_cols), R, loop_n)


def _plan(lens):
    """Pack layout: per-sample packed sizes/offsets and per-core quota."""
    s2 = lens.astype(np.int64) ** 2
    T = int(s2.sum())
    pbase = np.zeros(len(lens) + 1, np.int64)
    pbase[1:] = np.cumsum(s2)
    cpp = -(-T // (NCORES * P_DIM))  # ceil cols-per-partition per core
    sizes_cols = [int(cpp)]
    Q = P_DIM * cpp
    tprefix = np.array([0, Q], np.int64)
    return s2, T, pbase, Q, sizes_cols, tprefix


def _pack_stream(am, lens, T, pbase, Q):
    Pstream = np.zeros(NCORES * Q, dtype=np.float16)
    for i in range(len(lens)):
        s = int(lens[i])
        Pstream[pbase[i]:pbase[i + 1]].reshape(s, s)[...] = am[i, :s, :s]
    return Pstream


def _reorder_device_out(dev, cpp, R):
    """Device out buffer -> [R, Q] replica-major view matching the packed
    stream layout. Device layout: G groups of K replicas (one wide-row
    store each: group g at [g*K*Q, (g+1)*K*Q), partition-major rows of
    K*cpp), then REM replicas partition-major."""
    K, cppe = _layout(cpp, R)
    G = R // K
    REM = R - G * K
    Q = P_DIM * cpp
    c3 = K * cppe
    cpad = -(-c3 // 2048) * 2048
    cpad1 = -(-cpp // 2048) * 2048
    parts = []
    if G:
        main = dev[:G * P_DIM * cpad].reshape(G, P_DIM, cpad)
        main = main[:, :, 0:c3].reshape(G, P_DIM, K, cppe)[:, :, :, 0:cpp]
        parts.append(main.transpose(0, 2, 1, 3).reshape(G * K, Q))
    if REM:
        off0 = G * P_DIM * cpad
        rem = dev[off0:off0 + REM * P_DIM * cpad1]
        rem = rem.reshape(REM, P_DIM, cpad1)[:, :, 0:cpp]
        parts.append(rem.reshape(REM, Q))
    return np.concatenate(parts, axis=0) if len(parts) > 1 else parts[0]


def _assemble(outs, lens, s2, pbase, Q, tprefix, R):
    """outs[k]: [R, Q] replica-major per-core buffers; returns the final
    packed concat."""
    T = int(pbase[-1])
    F = np.empty(R * T, dtype=np.float16)
    core_cuts = np.array([k * Q for k in range(NCORES + 1)], np.int64)
    for i in range(len(lens)):
        sz = int(s2[i])
        g0 = int(pbase[i])
        g1 = g0 + sz
        inner = core_cuts[(core_cuts > g0) & (core_cuts < g1)]
        cuts = [g0] + [int(x) for x in inner] + [g1]
        for h in range(R):
            dst0 = R * g0 + h * sz
            for a, b in zip(cuts[:-1], cuts[1:]):
                k = a // Q
                loc = a - k * Q
                F[dst0 + (a - g0):dst0 + (b - g0)] = \
                    outs[k][h, loc:loc + (b - a)]
    return F


def kernel(attention_mask, seq_lengths, num_heads):
    am = np.asarray(attention_mask)
    if am.dtype != np.float16:
        am = am.astype(np.float16)
    lens = np.asarray(seq_lengths).astype(np.int64)
    R = int(np.asarray(num_heads))

    s2, T, pbase, Q, sizes_cols, tprefix = _plan(lens)
    if R == 0 or T == 0:
        return np.zeros(R * T, dtype=np.float16)
    Pstream = _pack_stream(am, lens, T, pbase, Q)
    in_maps = [{"inp": Pstream[k * Q:(k + 1) * Q]} for k in range(NCORES)]

    try:
        outs = _run_device(sizes_cols, R, in_maps)
    except Exception:
        # Transient device loss (NRT_EXEC_UNIT_UNRECOVERABLE): the terminal
        # self-recovers after a pause, but only a FRESH process can
        # reconnect — the in-process jax client stays wedged. Retry in
        # subprocesses.
        outs = _run_device_subprocess(in_maps, sizes_cols, R)
    cpp = sizes_cols[0]
    reps = [_reorder_device_out(d, cpp, R) for d in outs]
    return _assemble(reps, lens, s2, pbase, Q, tprefix, R)


def _run_device(sizes_cols, R, in_maps):
    from concourse.bass_utils import run_bass_kernel_spmd
    nc = _prod_nc(sizes_cols, R)
    res = run_bass_kernel_spmd(nc, in_maps, core_ids=list(range(NCORES)))
    return [np.asarray(res.results[k]["out"]).reshape(-1)
            for k in range(NCORES)]


def _subproc_entry(tmpdir):
    """Runs inside the retry subprocess: load staged inputs, run, save."""
    import os
    meta = np.load(os.path.join(tmpdir, "meta.npy"))
    R, ntiles = int(meta[0]), int(meta[1])
    sizes_cols = [int(x) for x in meta[2:2 + ntiles]]
    Pstream = np.load(os.path.join(tmpdir, "pstream.npy"))
    Q = Pstream.size // NCORES
    in_maps = [{"inp": Pstream[k * Q:(k + 1) * Q]} for k in range(NCORES)]
    outs = _run_device(sizes_cols, R, in_maps)
    np.save(os.path.join(tmpdir, "outs.npy"), np.stack(outs))


def _run_device_subprocess(in_maps, sizes_cols, R, attempts=3):
    import os
    import subprocess
    import sys
    import tempfile
    import time

    kdir = os.path.dirname(os.path.abspath(__file__))
    with tempfile.TemporaryDirectory() as td:
        meta = np.array([R, len(sizes_cols)] + list(sizes_cols),
                        np.int64)
        np.save(os.path.join(td, "meta.npy"), meta)
        Pstream = np.concatenate([m["inp"] for m in in_maps])
        np.save(os.path.join(td, "pstream.npy"), Pstream)
        code = (f"import sys; sys.path.insert(0, {kdir!r}); "
                f"import kernel; kernel._subproc_entry({td!r})")
        err = None
        for i in range(attempts):
            time.sleep(90 if i else 10)  # let the terminal recover first
            p = subprocess.run([sys.executable, "-c", code],
                               capture_output=True, text=True,
                               timeout=1800)
            if p.returncode == 0 and os.path.exists(
                    os.path.join(td, "outs.npy")):
                stacked = np.load(os.path.join(td, "outs.npy"))
                return [stacked[k] for k in range(NCORES)]
            err = p.stderr[-2000:]
        raise RuntimeError(f"device retries exhausted: {err}")
